# revision 27
# baseline (speedup 1.0000x reference)
"""Trainium2 Bass kernel for nn_ASC_LSTM (per-step LSTM encoder/decoder).

Strategy: data-parallel over batch (32 rows/core x 8 cores). Weights are
replicated, host-scaled by 64 and quantized to fp8 e3m4 (all 16-bit
on-chip surfaces use float16, whose 10-bit mantissa keeps the
recurrent-chain rounding error ~8x below bfloat16), then streamed from
HBM in 2-step chunks (one large DMA per chunk, double-buffered).
Gates are computed transposed ([gate_rows, batch] in PSUM) with all 16
gate chunks of a step accumulated into a single PSUM bank; the per-step
bias is folded in with one K=16 matmul against a one-hot "ones" tensor.
Gate order is [i, f, o, g] so one sigmoid covers chunks 0:12 and one
tanh covers 12:16 (PSUM read + 1/64 descale fused into the activation).
The elu is batched per 8 steps (its Exp needs a different activation
table than sigmoid/tanh); its "-1" is folded into the decoder bias on
the host via row sums of the quantized decoder weights, which is exact
because the skip blend coefficients sum to 1. The sequential skip-blend
chain is re-expressed as a running v-chain (computed as elu blocks
finish) plus 16 independent end corrections u'[4n] = v[n] +
2^-(n+1) * u[60], issued in descending n to match the decoder's
consumption order. Decoder weights/bias prefetch before the encoder
finishes so the DMA stream never idles at the phase boundary.
"""
import os
import sys

import numpy as np
import ml_dtypes

sys.path.insert(0, "/opt/trn_rl_repo")

import concourse.bass as bass
import concourse.tile as tile
from concourse import bacc, mybir
from concourse import bass_utils

B, I, H, S, RES = 256, 256, 512, 64, 4
NCORES = 8
BLOC = B // NCORES  # 32
ECH = 1  # encoder steps per weight-DMA chunk
DCH = 4  # decoder idxs per weight-DMA chunk
WSCALE = 64.0
F16 = mybir.dt.float16
F32 = mybir.dt.float32
FP8 = mybir.dt.float8e3
AF = mybir.ActivationFunctionType

_STATE = {}


def _build_module():
    nc = bacc.Bacc(
        "TRN2",
        target_bir_lowering=False,
        debug=False,
        enable_asserts=False,
        num_devices=NCORES,
    )
    wt_d = nc.dram_tensor("wt", [128, S, 6, 16, 128], FP8, kind="ExternalInput").ap()
    wdt_d = nc.dram_tensor("wdt", [128, S, 4, 6, 128], FP8, kind="ExternalInput").ap()
    x_d = nc.dram_tensor("xr", [128, S, 2, BLOC], F16, kind="ExternalInput").ap()
    benc_d = nc.dram_tensor("benc", [16, S, 128], F16, kind="ExternalInput").ap()
    bdec_d = nc.dram_tensor("bdec", [12, S // 2, 128], F16, kind="ExternalInput").ap()
    eones_d = nc.dram_tensor("eones", [16, 16, BLOC], F16, kind="ExternalInput").ap()
    dones_d = nc.dram_tensor("dones", [12, 6, 2, BLOC], F16, kind="ExternalInput").ap()
    out_d = nc.dram_tensor("out", [128, S, 2, BLOC], F16, kind="ExternalOutput").ap()

    inv = 1.0 / WSCALE

    with tile.TileContext(nc) as tc:
        with (
            tc.tile_pool(name="wenc", bufs=3) as wpool,
            tc.tile_pool(name="wdec", bufs=7) as wdpool,
            tc.tile_pool(name="big", bufs=1) as bigpool,
            tc.tile_pool(name="gates", bufs=2) as gpool,
            tc.tile_pool(name="small", bufs=2) as spool,
            tc.tile_pool(name="psum", bufs=4, space="PSUM") as psum,
        ):
            x_sb = bigpool.tile([128, S, 2, BLOC], F16, tag="xsb")
            nc.sync.dma_start(out=x_sb, in_=x_d)
            benc_sb = bigpool.tile([16, S, 128], F16, tag="benc")
            nc.sync.dma_start(out=benc_sb, in_=benc_d)
            eones_sb = bigpool.tile([16, 16, BLOC], F16, tag="eones")
            nc.sync.dma_start(out=eones_sb, in_=eones_d)
            dones_sb = bigpool.tile([12, 6, 2, BLOC], F16, tag="dones")
            nc.sync.dma_start(out=dones_sb, in_=dones_d)
            bdec_sb = bigpool.tile([12, S // 2, 128], F16, tag="bdec")
            nc.sync.dma_start(out=bdec_sb, in_=bdec_d)

            # f16 h history; becomes u = elu(h)+1 in place, then blended.
            hist = bigpool.tile([128, S, 4, BLOC], F16, tag="hist")
            vtile = bigpool.tile([128, S // RES, 4, BLOC], F16, tag="vt")
            out_sb = bigpool.tile([128, S, 2, BLOC], F16, tag="outsb")

            dec_w = {}

            # ---------------- encoder scan ----------------
            # the whole recurrence chain runs in f16 (DVE 2x mode); the next
            # step reads hist[:, t-1] directly, and elu blocks are delayed by
            # one step so they never overwrite a slot the next step still
            # needs
            def elu_block(t0, size, ks):
                blk = hist[:, t0 : t0 + size]
                en_full = spool.tile([128, 8, 4, BLOC], F16, tag="eneg")
                en = en_full[:, :size]
                nc.vector.tensor_scalar_min(en, blk, 0.0)
                nc.vector.tensor_scalar_max(blk, blk, 0.0)
                nc.scalar.activation(out=en, in_=en, func=AF.Exp)
                nc.vector.tensor_add(blk, blk, en)
                # v-chain updates for blend positions now available:
                # v[n] = (u[4n] + v[n-1])/2
                for k in ks:
                    n = k // RES
                    if n == 0:
                        nc.vector.tensor_scalar_mul(vtile[:, 0], hist[:, 0], 0.5)
                    else:
                        nc.vector.tensor_add(vtile[:, n], hist[:, k], vtile[:, n - 1])
                        nc.vector.tensor_scalar_mul(vtile[:, n], vtile[:, n], 0.5)

            for c in range(S // ECH):
                w_sb = wpool.tile([128, ECH, 6, 16, 128], FP8, tag="w")
                nc.sync.dma_start(out=w_sb, in_=wt_d[:, c * ECH : (c + 1) * ECH])
                for i in range(ECH):
                    t = c * ECH + i
                    ps = psum.tile([128, 16, BLOC], F32, tag="ps")
                    nc.tensor.matmul(
                        ps, lhsT=benc_sb[:, t], rhs=eones_sb,
                        start=True, stop=False, skip_group_check=True,
                    )
                    # x-dependent matmuls first: the PE queue is in-order, so
                    # issuing these before the h-matmuls lets the PE work
                    # while the previous step's h is still being produced
                    for m in range(16):
                        for k in range(2):
                            nc.tensor.matmul(
                                ps[:, m], lhsT=w_sb[:, i, k, m], rhs=x_sb[:, t, k],
                                start=False, stop=(t == 0 and k == 1),
                                skip_group_check=True,
                            )
                    if t > 0:
                        # g-gate chunks (12:16) first so the tanh activation
                        # overlaps the remaining h-matmuls
                        for m in (12, 13, 14, 15, 0, 1, 2, 3, 4, 5, 6, 7, 8, 9, 10, 11):
                            for k in range(2, 6):
                                nc.tensor.matmul(
                                    ps[:, m], lhsT=w_sb[:, i, k, m], rhs=hist[:, t - 1, k - 2],
                                    start=False, stop=(k == 5),
                                    skip_group_check=True,
                                )
                    gs = gpool.tile([128, 16, BLOC], F16, tag="gs")
                    nc.scalar.activation(out=gs[:, 12:16], in_=ps[:, 12:16], func=AF.Tanh, scale=inv)
                    nc.scalar.activation(out=gs[:, 0:12], in_=ps[:, 0:12], func=AF.Sigmoid, scale=inv)
                    # c = f*h_prev + i*g ; h = o*tanh(c)
                    cc = spool.tile([128, 4, BLOC], F16, tag="cc")
                    nc.vector.tensor_mul(cc, gs[:, 0:4], gs[:, 12:16])
                    if t > 0:
                        fh = spool.tile([128, 4, BLOC], F16, tag="fh")
                        nc.vector.tensor_mul(fh, gs[:, 4:8], hist[:, t - 1])
                        nc.vector.tensor_add(cc, cc, fh)
                    tct = spool.tile([128, 4, BLOC], F16, tag="tct")
                    nc.scalar.activation(out=tct, in_=cc, func=AF.Tanh)
                    nc.vector.tensor_mul(hist[:, t], tct, gs[:, 8:12])
                    # delayed batched elu: u = relu(h) + exp(min(h,0)), the
                    # -1 is folded into the decoder bias on host
                    if t % 8 == 0 and t > 0:
                        elu_block(t - 8, 8, (t - 8, t - 4))
                    elif t == S - 4:
                        elu_block(S - 8, 4, (S - 8,))
            # finish the elu fine-grained: t=62..63 first so the decoder's
            # first pair (tsrc 63, 62) starts while 60..61 processes
            elu_block(S - 2, 2, ())
            elu_block(S - 4, 2, (S - 4,))

            # ---------------- skip blend end corrections ----------------
            # u'[4n] = v[n] + 2^-(n+1) * u[60]; descending n matches the
            # decoder's consumption order (idx 4j+3 reads t = 60-4j).
            u60 = spool.tile([128, 4, BLOC], F16, tag="u60")
            nc.vector.tensor_copy(out=u60, in_=hist[:, S - RES])
            for n in range(S // RES - 1, -1, -1):
                bc = spool.tile([128, 4, BLOC], F16, tag="bc")
                nc.vector.tensor_scalar_mul(bc, u60, 0.5 ** (n + 1))
                nc.vector.tensor_add(hist[:, n * RES], vtile[:, n], bc)

            # ---------------- decoder (parallel over idx, 2 idx/batch) ----
            rn_prev = None
            for c in range(S // DCH):
                if c in dec_w:
                    wd_sb = dec_w.pop(c)
                else:
                    wd_sb = wdpool.tile([128, DCH, 4, 6, 128], FP8, tag="wd")
                    nc.sync.dma_start(out=wd_sb, in_=wdt_d[:, c * DCH : (c + 1) * DCH])
                for jp in range(DCH // 2):
                    i0 = c * DCH + jp * 2  # idx pair (i0, i0+1)
                    psd = psum.tile([128, 6, 2, BLOC], F32, tag="psd")
                    nc.tensor.matmul(
                        psd, lhsT=bdec_sb[:, i0 // 2], rhs=dones_sb,
                        start=True, stop=False, skip_group_check=True,
                    )
                    for m in range(6):
                        for j in range(2):
                            tsrc = S - 1 - (i0 + j)
                            for k in range(4):
                                nc.tensor.matmul(
                                    psd[:, m, j],
                                    lhsT=wd_sb[:, jp * 2 + j, k, m],
                                    rhs=hist[:, tsrc, k],
                                    start=False, stop=(k == 3),
                                    skip_group_check=True,
                                )
                    gd = gpool.tile([128, 6, 2, BLOC], F16, tag="gd")
                    nc.scalar.activation(out=gd[:, 0:4], in_=psd[:, 0:4], func=AF.Sigmoid, scale=inv)
                    nc.scalar.activation(out=gd[:, 4:6], in_=psd[:, 4:6], func=AF.Tanh, scale=inv)
                    cd = spool.tile([128, 2, 2, BLOC], F16, tag="cd")
                    nc.vector.tensor_mul(cd, gd[:, 0:2], gd[:, 4:6])
                    nc.scalar.activation(out=cd, in_=cd, func=AF.Tanh)
                    nc.vector.tensor_mul(cd, cd, gd[:, 2:4])  # hd, [128, hh, j, b]
                    hdT = cd.transpose([0, 2, 1, 3])  # [128, j, hh, b] view
                    rn = spool.tile([128, 2, 2, BLOC], F16, tag="rn")
                    if i0 % RES == 0:
                        nc.vector.tensor_copy(out=rn[:, 0], in_=hdT[:, 0])
                    else:
                        nc.vector.tensor_add(rn[:, 0], rn_prev[:, 1], hdT[:, 0])
                    nc.vector.tensor_add(rn[:, 1], rn[:, 0], hdT[:, 1])
                    rn_prev = rn
                    nc.scalar.activation(out=out_sb[:, i0 : i0 + 2], in_=rn, func=AF.Tanh)
                if c % 4 == 3 and c < 12:
                    s0 = (c - 3) * DCH
                    nc.sync.dma_start(
                        out=out_d[:, s0 : s0 + 16], in_=out_sb[:, s0 : s0 + 16]
                    )
                elif c in (13, 14):
                    s0 = c * DCH - 4
                    nc.sync.dma_start(
                        out=out_d[:, s0 : s0 + 8], in_=out_sb[:, s0 : s0 + 8]
                    )
                elif c == 15:
                    nc.sync.dma_start(
                        out=out_d[:, 60:64], in_=out_sb[:, 60:64]
                    )
    nc.finalize()
    return nc


def _host_prep(inputs):
    f16 = np.float16
    f8 = ml_dtypes.float8_e3m4
    # encoder: gate order [i, f, o, g]
    eperm = np.r_[0:512, 512:1024, 1536:2048, 1024:1536]
    W_all = np.concatenate([inputs["Wih_enc"], inputs["Whh_enc"]], axis=2)[:, eperm, :] * WSCALE
    # [t, 16m, 128q, 6k, 128p] -> [p, t, k, m, q]
    wt = np.ascontiguousarray(
        W_all.reshape(S, 16, 128, 6, 128).transpose(4, 0, 3, 1, 2)
    ).astype(f8)
    benc = np.ascontiguousarray(
        ((inputs["bih_enc"] + inputs["bhh_enc"])[:, eperm] * WSCALE)
        .reshape(S, 16, 128)
        .transpose(1, 0, 2)
    ).astype(f16)
    eones = np.ascontiguousarray(
        np.repeat(np.eye(16, dtype=np.float32)[:, :, None], BLOC, axis=2)
    ).astype(f16)
    # decoder: gate order [i, o, g]
    dperm = np.r_[0:256, 768:1024, 512:768]
    Wd = inputs["Wih_dec"][:, dperm, :] * WSCALE
    wd8 = np.ascontiguousarray(
        Wd.reshape(S, 6, 128, 4, 128).transpose(4, 0, 3, 1, 2)  # [p,t,k,m,q]
    ).astype(f8)
    # fold elu's "-1" into the bias: subtract row sums of the quantized W
    corr = wd8.astype(np.float32).sum(axis=(0, 2))  # [t, m, q]
    bd = ((inputs["bih_dec"] + inputs["bhh_dec"])[:, dperm] * WSCALE).reshape(S, 6, 128) - corr
    # idx-pair packing: bdec[(m*2+j), pair, q] = bd[2*pair+j, m, q]
    bdec = np.ascontiguousarray(
        bd.reshape(S // 2, 2, 6, 128).transpose(2, 1, 0, 3).reshape(12, S // 2, 128)
    ).astype(f16)
    dones = np.ascontiguousarray(
        np.repeat(
            np.eye(12, dtype=np.float32).reshape(12, 6, 2)[:, :, :, None], BLOC, axis=3
        )
    ).astype(f16)
    xr = np.ascontiguousarray(
        inputs["x"].reshape(B, 2, 128, S).transpose(2, 3, 1, 0)
    ).astype(f16)
    return wt, benc, eones, wd8, bdec, dones, xr


def kernel(**inputs):
    inputs = {k: np.asarray(v) for k, v in inputs.items()}
    if "nc" not in _STATE:
        _STATE["nc"] = _build_module()
    nc = _STATE["nc"]
    wt, benc, eones, wdt, bdec, dones, xr = _host_prep(inputs)
    in_maps = []
    for c in range(NCORES):
        in_maps.append(
            {
                "wt": wt,
                "wdt": wdt,
                "benc": benc,
                "bdec": bdec,
                "eones": eones,
                "dones": dones,
                "xr": np.ascontiguousarray(xr[:, :, :, c * BLOC : (c + 1) * BLOC]),
            }
        )
    res = bass_utils.run_bass_kernel_spmd(
        nc,
        in_maps,
        core_ids=list(range(NCORES)),
        trace=bool(int(os.environ.get("BASS_KERNEL_TRACE", "0"))),
    )
    _STATE["last_results"] = res
    outs = []
    for c in range(NCORES):
        o = np.asarray(res.results[c]["out"]).astype(np.float32)  # [128, S, 2, BLOC]
        outs.append(
            np.ascontiguousarray(
                o.transpose(3, 2, 0, 1).reshape(BLOC, 2 * 128, S)[:, :, ::-1]
            )
        )
    return np.concatenate(outs, axis=0).astype(np.float32)
